# revision 32
# baseline (speedup 1.0000x reference)
"""Trainium2 Bass kernel for nn_BalanceLabelAugmentation2 (topk_masking).

Math (reference, restructured):
  Z   = feat @ W.T            [N, 51]   (matmul is linear over the mixup!)
  lo  = feat_u @ W_o.T + b_o  [N_u, 51] -> pred=argmax, score=max softmax
  midw_i  = gm[pred_i] & (score_i > 0.5);  tailw_i = gt[pred_i] & (score_i > 0.3)
  For pair (copy c, unlabeled row i) with partner j = idx_c[i]:
    l    = 0.7*Z_o[j] + 0.3*Z_u[i] + b
    ce   = logsumexp(l) - 0.7*l[label_j] - 0.3*l[pred_i]
  out = sum(ce*w) / max(sum w, 1)

NO-COLLECTIVE design (v6).  Every core receives a full local copy of the
labeled FEATURES (+0.7*onehot(label)) as a DRAM gather table; zero
collectives.  The device ships per-slot [sum(l*sel) | sum(exp l)] rows
plus the compaction counts and the dense weight sum; the host (which
already sums the per-core partials) finishes with ln() on the <=320
valid slots per core.

Per core:
  Phase B   fp8 DoubleRow matmul of the unlabeled shard with
            [0.3*W | W_o] heads (feat*8 and weights*8 host scales, PSUM
            descale 1/64), one [115,128] PE transpose per 128-row chunk,
            and PER-TILE mask passes that overlap the next tile's
            matmul.  zu rows (256 bf16 elems) carry [0.3*Zu | j_hi | ...
            | 0.3*OHpred | j_lo] where j_hi/j_lo split the partner idx
            into bf16-exact 7-bit halves (partitions 64:69, chunks 0/1).
  Compact   TWO sparse_gathers (mid row-ids, tail row-ids) -> counts;
            handles arbitrary (even overlapping) group masks exactly.
  Gathers   zu idx staged via a replication MATMUL; transpose-mode zu
            gather (slots 0:64 mid, 64:128 tail); partner j rebuilt from
            the gathered hi/lo cols and staged DMA-FREE into the 16-lane
            idx layout (transpose + 4 lane-select matmuls + replication
            matmul); transpose-mode ftab gather (384 idx, 320 live).
  CE        [class, slot]: l = 8-chunk 0.7*W matmul + zu broadcast (+b);
            sel = 0.7*OHlab + 0.3*OHpred; per-slot class sums via two
            ones-matmuls -> shipped to host.
"""

import numpy as np
import ml_dtypes

import concourse.bass as bass
import concourse.tile as tile
from concourse import bacc, mybir
from concourse.bass_utils import run_bass_kernel_spmd
from concourse.tile_rust import add_dep_helper

F32 = mybir.dt.float32
BF16 = mybir.dt.bfloat16
FP8 = mybir.dt.float8e4
XSCALE = 8.0   # host feat scale (avoids e4m3 subnormals)
WSCALE = 8.0   # host weight scale; PSUM descale = 1/(XSCALE*WSCALE)
I16 = mybir.dt.int16
U32 = mybir.dt.uint32
AF = mybir.ActivationFunctionType
ALU = mybir.AluOpType
AX = mybir.AxisListType

KU = 64                # per-group survivor capacity (worst observed ~27)
NV = 5 * KU            # live ftab slots (320); slot k = 64c + s
NT = 6 * KU            # ftab gather idx count incl pad block (%128==0)
NW = NT // 16          # staged idx words (24)
NOUT = 2 * NV + 4      # [red0 | red1 | wsum, nm, nt, pad]


class Cfg:
    def __init__(self, n_o=16384, n_u=16384, d=1024, cores=8, rowt=512):
        self.n_o, self.n_u, self.d, self.cores, self.rowt = n_o, n_u, d, cores, rowt
        self.c = 51
        self.u = n_u // cores          # unlabeled rows per core
        self.kc = d // 128             # contraction chunks
        self.unl_tiles = self.u // rowt
        self.cpt = rowt // 128         # 128-row chunks per tile
        self.chunks = self.u // 128    # unlabeled 128-row chunks
        self.frow = d + 256            # fp8 ftab row bytes (feat + onehot)
        self.zrow = 256                # zu row bf16 elems (512B)
        assert self.u % rowt == 0 and d % 128 == 0
        assert self.chunks <= 16


def _bc(tile_ap, offset_ap, pattern):
    """AP on tile_ap's tensor at offset_ap's offset with a custom free pattern."""
    return bass.AP(tensor=tile_ap.tensor, offset=offset_ap.offset,
                   ap=[tile_ap.ap[0]] + pattern)


def build_bass(cfg: Cfg, use_bias: bool):
    C, FROW, ZROW, KC, ROWT = cfg.c, cfg.frow, cfg.zrow, cfg.kc, cfg.rowt
    WTC = 128  # [0.3W | pad | W_o | pad], fp8 dual-weights want even cols
    CH = cfg.chunks
    nc = bacc.Bacc("TRN2", target_bir_lowering=False, debug=False,
                   num_devices=cfg.cores)

    xu_h = nc.dram_tensor("xu", [cfg.unl_tiles, 128, KC, ROWT], FP8,
                          kind="ExternalInput")
    wtu_h = nc.dram_tensor("wtu", [128, KC, WTC], FP8, kind="ExternalInput")
    wtl_h = nc.dram_tensor("wtl", [128, KC, 64], FP8, kind="ExternalInput")
    ftab_h = nc.dram_tensor("ftab", [cfg.n_o, FROW], FP8,
                            kind="ExternalInput")
    # consts cols: gm[0:C], gt[C:2C], iotarow_p1[2C : 2C+CH],
    # idx_hi [.. 5CH], idx_lo [.. 5CH], repmat [.. 128], identity [.. 128],
    # fused lane-select E'_s1[s, p] = (s == 16*s1 + p%16)  [.. 4*128]
    O_GM, O_GT = 0, C
    O_RP1 = 2 * C
    O_HI = O_RP1 + CH
    O_LO = O_HI + 5 * CH
    O_REP = O_LO + 5 * CH
    O_ID = O_REP + 128
    O_ES = O_ID + 128
    NCONST = O_ES + 4 * 128
    consts_h = nc.dram_tensor("consts", [128, NCONST], F32, kind="ExternalInput")
    biascol_h = nc.dram_tensor("biascol", [WTC, 2], F32, kind="ExternalInput")
    out_h = nc.dram_tensor("out", [1, NOUT], F32, kind="ExternalOutput")

    with tile.TileContext(nc) as tc:
        ppcm = tc.tile_pool(name="persist", bufs=1)
        pp_ = ppcm.__enter__()

        def P(shape, dtype, name):
            return pp_.tile(shape, dtype, name=name, tag=name)

        wtu_sb = P([128, KC, WTC], FP8, "wtu_sb")
        wtl_sb = P([128, KC, 64], FP8, "wtl_sb")
        consts_sb = P([128, NCONST], F32, "consts_sb")
        gm_r = consts_sb[:, O_GM:O_GM + C]
        gt_r = consts_sb[:, O_GT:O_GT + C]
        iotarow_p1 = consts_sb[:, O_RP1:O_RP1 + CH]
        repmat = consts_sb[0:16, O_REP:O_REP + 128]
        ident = consts_sb[:, O_ID:O_ID + 128]
        if use_bias:
            biascol_sb = P([WTC, 2], F32, "biascol_sb")
            nc.sync.dma_start(out=biascol_sb[:], in_=biascol_h[:])
        ones128 = P([128, 1], F32, "ones128")
        nc.vector.memset(ones128[:], 1.0)
        ones_bf = P([128, 1], BF16, "ones_bf")
        nc.vector.memset(ones_bf[:], 1.0)

        zu_all = P([128, CH, ZROW], BF16, "zu_all")
        lo_all = P([128, CH, C], F32, "lo_all")
        wbuf = P([128, 2, CH], F32, "wbuf")
        vpack = P([128, 2, CH], F32, "vpack")
        vt_sb = P([16, 2, 128], F32, "vt_sb")
        sgMT = P([16, 8], F32, "sgMT")           # [mid | tail] rowids
        cnt8 = P([1, 8], U32, "cnt8")
        cntf = P([1, 2], F32, "cntf")
        stgBf = P([16, 8], F32, "stgBf")
        idx_sbB = P([128, 8], I16, "idx_sbB")
        jf128 = P([128, KU], F32, "jf128")       # j rebuilt at parts 64:69
        jT_sb = P([KU, 8], F32, "jT_sb")         # transposed j [s, c]
        stg16 = P([16, NW], F32, "stg16")        # lane-word wrapped j
        nc.vector.memset(stg16[:], 0.0)          # pad words 20:24 stay 0
        idx_sbA = P([128, NW], I16, "idx_sbA")
        nc.vector.memset(idx_sbA[:, 20:NW], 0)
        gtm = P([128, KC + 2, NT], FP8, "gtm")   # transposed fp8 ftab
        zuT = P([128, 2, 128], BF16, "zuT")       # transpose-gathered zu
        l_sb = P([C, NV], F32, "l_sb")
        esel = P([C, 2, NV], BF16, "esel")        # [l*sel | e]
        wcol = P([128, 1], F32, "wcol")
        outrow = P([1, NOUT], F32, "outrow")
        nc.vector.memset(outrow[0:1, NOUT - 1:NOUT], 0.0)

        tsem = nc.alloc_semaphore("tsem")
        zsem = nc.alloc_semaphore("zsem")

        with tc.tile_pool(name="dramp", bufs=1, space="DRAM") as dramp:
            zu_dram = dramp.tile([cfg.u, ZROW], BF16, name="zu_dram")

            with (
                tc.tile_pool(name="xt", bufs=cfg.unl_tiles) as xt_pool,
                tc.tile_pool(name="ztp", bufs=2, space="PSUM") as zt_pool,
                tc.tile_pool(name="zts", bufs=4) as zts_pool,
                tc.tile_pool(name="trp", bufs=3, space="PSUM") as tr_pool,
                tc.tile_pool(name="vtpp", bufs=1, space="PSUM") as vtp_pool,
                tc.tile_pool(name="ppp", bufs=1, space="PSUM") as pp_pool,
                tc.tile_pool(name="lpp", bufs=1, space="PSUM") as lp_pool,
                tc.tile_pool(name="small", bufs=8) as small_pool,
                tc.tile_pool(name="stat", bufs=24) as stat_pool,
            ):
                def xtile_load(t):
                    xt = xt_pool.tile([128, KC, ROWT], FP8, name="xt",
                                      tag="xtu")
                    nc.scalar.dma_start(
                        out=xt[:],
                        in_=bass.AP(tensor=xu_h, offset=t * 128 * KC * ROWT,
                                    ap=[[KC * ROWT, 128], [ROWT, KC],
                                        [1, ROWT]]))
                    return xt

                nc.sync.dma_start(out=wtu_sb[:], in_=wtu_h[:])
                xus = [xtile_load(t) for t in range(cfg.unl_tiles)]
                nc.sync.dma_start(out=wtl_sb[:], in_=wtl_h[:])
                nc.sync.dma_start(out=consts_sb[:], in_=consts_h[:])
                # partner idx halves into the zu rows (bf16-exact 7-bit):
                # mid copies at row elems 64:66 / 192:194, tail copies at
                # 96:99 / 224:227 (32-aligned partition bases post-gather)
                for eo, co, ncp in ((64, 0, 2), (96, 2 * CH, 3)):
                    nc.vector.tensor_copy(
                        _bc(zu_all[:], zu_all[:, 0:1, eo:eo + 1],
                            [[ZROW, CH], [1, ncp]]),
                        _bc(consts_sb[:], consts_sb[:, O_HI + co:O_HI + co + 1],
                            [[1, CH], [CH, ncp]]))
                    nc.vector.tensor_copy(
                        _bc(zu_all[:], zu_all[:, 0:1, 128 + eo:128 + eo + 1],
                            [[ZROW, CH], [1, ncp]]),
                        _bc(consts_sb[:], consts_sb[:, O_LO + co:O_LO + co + 1],
                            [[1, CH], [CH, ncp]]))

                def matmul_tile_fp8(xt):
                    zt = zt_pool.tile([WTC, ROWT], F32, tag="zt", name="zt")
                    for k in range(0, KC, 2):
                        nc.tensor.matmul(
                            zt[:], lhsT=wtu_sb[:, k:k + 2, :],
                            rhs=xt[:, k:k + 2, :], start=(k == 0),
                            stop=(k == KC - 2),
                            perf_mode=mybir.MatmulPerfMode.DoubleRow)
                    return zt

                # ========== Phase B + per-tile masks (overlapped) ==========
                for t in range(cfg.unl_tiles):
                    zt = matmul_tile_fp8(xus[t])
                    zq = zts_pool.tile([WTC, ROWT], F32, tag="zq", name="zq")
                    nc.scalar.activation(
                        zq[:], zt[:], AF.Copy,
                        bias=(biascol_sb[:, 1:2] if use_bias else 0.0),
                        scale=1.0 / (XSCALE * WSCALE))
                    for q in range(cfg.cpt):
                        g = t * cfg.cpt + q
                        qs = slice(q * 128, (q + 1) * 128)
                        tr = tr_pool.tile([128, 64 + C], F32, tag="tr",
                                          name="tr")
                        nc.tensor.transpose(tr[:], zq[0:64 + C, qs],
                                            ident[0:64 + C, 0:64 + C])
                        nc.scalar.copy(zu_all[:, g, 0:C], tr[:, 0:C])
                        nc.scalar.copy(lo_all[:, g, :], tr[:, 64:64 + C])

                    # masks for this tile's chunks [128, cpt, C]
                    sl = slice(t * cfg.cpt, (t + 1) * cfg.cpt)
                    lo_t = lo_all[:, sl, :]
                    TP = cfg.cpt
                    mx = stat_pool.tile([128, TP], F32, tag="st", name="mx")
                    nc.vector.tensor_reduce(mx[:], lo_t, axis=AX.X,
                                            op=ALU.max)
                    ew = small_pool.tile([128, TP, C], F32, tag="ew",
                                         name="ew")
                    nc.scalar.activation(ew[:], lo_t, AF.Exp)
                    ssum = stat_pool.tile([128, TP], F32, tag="st",
                                          name="ssum")
                    nc.vector.tensor_reduce(ssum[:], ew[:], axis=AX.X,
                                            op=ALU.add)
                    em = stat_pool.tile([128, TP], F32, tag="st", name="em")
                    nc.scalar.activation(em[:], mx[:], AF.Exp)
                    mx_bc = _bc(mx[:], mx[:], [[1, TP], [0, C]])
                    oh = small_pool.tile([128, TP, C], F32, tag="oh",
                                         name="oh")
                    nc.vector.tensor_tensor(out=oh[:], in0=lo_t, in1=mx_bc,
                                            op=ALU.is_equal)
                    # 0.3*onehot(pred) into the zu rows (bf16 cast on write)
                    nc.vector.tensor_scalar_mul(
                        zu_all[:, sl, 128:128 + C], oh[:], 0.3)
                    gm_bc = _bc(consts_sb[:], gm_r, [[0, TP], [1, C]])
                    gt_bc = _bc(consts_sb[:], gt_r, [[0, TP], [1, C]])
                    jm = small_pool.tile([128, TP, C], F32, tag="ew",
                                         name="jm")
                    nc.vector.tensor_tensor(out=jm[:], in0=oh[:], in1=gm_bc,
                                            op=ALU.mult)
                    gvm = stat_pool.tile([128, TP], F32, tag="st", name="gvm")
                    nc.vector.tensor_reduce(gvm[:], jm[:], axis=AX.X,
                                            op=ALU.add)
                    jt = small_pool.tile([128, TP, C], F32, tag="oh",
                                         name="jt")
                    nc.vector.tensor_tensor(out=jt[:], in0=oh[:], in1=gt_bc,
                                            op=ALU.mult)
                    gvt = stat_pool.tile([128, TP], F32, tag="st", name="gvt")
                    nc.vector.tensor_reduce(gvt[:], jt[:], axis=AX.X,
                                            op=ALU.add)
                    # score>thr  <=>  thr*sum(e) < e^max
                    tm = stat_pool.tile([128, TP], F32, tag="st", name="tm")
                    nc.vector.scalar_tensor_tensor(
                        out=tm[:], in0=ssum[:], scalar=0.5, in1=em[:],
                        op0=ALU.mult, op1=ALU.is_lt)
                    nc.vector.tensor_tensor(out=wbuf[:, 0, sl], in0=tm[:],
                                            in1=gvm[:], op=ALU.mult)
                    tt = stat_pool.tile([128, TP], F32, tag="st", name="tt")
                    nc.vector.scalar_tensor_tensor(
                        out=tt[:], in0=ssum[:], scalar=0.3, in1=em[:],
                        op0=ALU.mult, op1=ALU.is_lt)
                    nc.vector.tensor_tensor(out=wbuf[:, 1, sl], in0=tt[:],
                                            in1=gvt[:], op=ALU.mult)
                    # v-list cols: v = (rowid+1)*w - 1 for mid and tail
                    for li in range(2):
                        nc.vector.tensor_tensor(
                            out=vpack[:, li, sl], in0=wbuf[:, li, sl],
                            in1=iotarow_p1[:, sl], op=ALU.mult)
                        nc.vector.tensor_scalar_add(vpack[:, li, sl],
                                                    vpack[:, li, sl], -1.0)
                    # zu rows of this tile -> DRAM (overlaps next tile)
                    zwr = nc.sync.dma_start(
                        out=bass.AP(tensor=zu_dram[:].tensor,
                                    offset=zu_dram[:].offset
                                    + t * ROWT * ZROW,
                                    ap=[[ZROW, 128], [128 * ZROW, TP],
                                        [1, ZROW]]),
                        in_=zu_all[:, sl, :])
                    if t == cfg.unl_tiles - 1:
                        zu_wr = zwr

                # ===== compaction: 2 sparse_gathers (mid/tail rowids) =====
                with tc.high_priority(offset=400):
                    for li in range(2):
                        pool = vtp_pool if li == 0 else pp_pool
                        vt_ps = pool.tile([16, 128], F32, tag="vtp",
                                          name="vt_ps")
                        nc.tensor.transpose(vt_ps[:], vpack[:, li, :],
                                            ident[:])
                        nc.vector.tensor_copy(vt_sb[:, li, :], vt_ps[:])
                    for li in range(2):
                        nc.gpsimd.sparse_gather(
                            out=sgMT[0:16, li * 4:(li + 1) * 4],
                            in_=vt_sb[:, li, :],
                            num_found=cnt8[0:1, li:li + 1])
                    nc.vector.tensor_copy(cntf[:], cnt8[0:1, 0:2])

                    # zu idx, replicated to all 8 gpsimd sub-core
                    # blocks via matmul; slots 0:64 mid, 64:128 tail
                    nc.vector.tensor_scalar(
                        out=stgBf[:], in0=sgMT[:],
                        scalar1=0.0, scalar2=float(cfg.u - 1),
                        op0=ALU.max, op1=ALU.min)
                    rp_ps = pp_pool.tile([128, 8], F32, tag="vtp",
                                         name="rp_ps")
                    nc.tensor.matmul(rp_ps[:], lhsT=repmat, rhs=stgBf[:],
                                     start=True, stop=True)
                    nc.vector.tensor_copy(idx_sbB[:], rp_ps[:])

                    gB = nc.gpsimd.dma_gather(
                        out_ap=zuT[:], in_ap=zu_dram[:],
                        idxs_ap=idx_sbB[:],
                        num_idxs=128, num_idxs_reg=128, elem_size=ZROW,
                        transpose=True)
                    gB.then_inc(zsem, 16)
                    add_dep_helper(gB.ins, zu_wr.ins, sync=True,
                                   reason="zu gather reads zu_dram")

                    wtV2 = nc.vector.wait_ge(zsem, 16)

                    # rebuild partner j = 128*hi + lo (mid at parts 64:66
                    # over slots 0:64, tail at parts 96:99 over 64:128)
                    jb1 = nc.vector.scalar_tensor_tensor(
                        out=jf128[64:66, :], in0=zuT[64:66, 0, 0:KU],
                        scalar=128.0, in1=zuT[64:66, 1, 0:KU],
                        op0=ALU.mult, op1=ALU.add)
                    add_dep_helper(jb1.ins, wtV2.ins, sync=False,
                                   reason="j cols land with zu gather")
                    nc.vector.scalar_tensor_tensor(
                        out=jf128[32:35, :], in0=zuT[96:99, 0, KU:128],
                        scalar=128.0, in1=zuT[96:99, 1, KU:128],
                        op0=ALU.mult, op1=ALU.add)
                    # DMA-free staging into the gather idx layout
                    # (slot k=64c+s -> lane s%16, word 4c+s//16):
                    # transpose j -> [s, c], 4 lane-select matmuls
                    # staged[l, 4c+s1] = jT[16*s1+l, c], one replication
                    # matmul to 128 partitions, clamp+cast to i16.
                    jt_ps = tr_pool.tile([KU, 8], F32, tag="tr",
                                         name="jt_ps")
                    nc.tensor.transpose(jt_ps[0:KU, 0:2], jf128[64:66, :],
                                        ident[64:66, 64:66])
                    nc.tensor.transpose(jt_ps[0:KU, 2:5], jf128[32:35, :],
                                        ident[32:35, 32:35])
                    nc.vector.tensor_copy(jT_sb[:, 0:5], jt_ps[0:KU, 0:5])
                    # fused lane-select + 8x sub-core replication:
                    # idx_sbA[p, 4c+s1] = jT[16*s1 + p%16, c]
                    for s1 in range(4):
                        es_ps = vtp_pool.tile([128, 8], F32, tag="vtp",
                                              name="es_ps")
                        nc.tensor.matmul(
                            es_ps[0:128, 0:5],
                            lhsT=consts_sb[0:KU, O_ES + 128 * s1:
                                           O_ES + 128 * s1 + 128],
                            rhs=jT_sb[:, 0:5], start=True, stop=True)
                        nc.vector.tensor_scalar(
                            out=_bc(idx_sbA[:], idx_sbA[:, s1:s1 + 1],
                                    [[4, 5]]),
                            in0=es_ps[0:128, 0:5],
                            scalar1=0.0, scalar2=float(cfg.n_o - 1),
                            op0=ALU.max, op1=ALU.min)

                    gA = nc.gpsimd.dma_gather(
                        out_ap=gtm[:], in_ap=ftab_h[:],
                        idxs_ap=idx_sbA[:],
                        num_idxs=NT, num_idxs_reg=NT, elem_size=FROW,
                        transpose=True)
                    gA.then_inc(tsem, 16)

                    wtT1 = nc.tensor.wait_ge(tsem, 16)
                    wt1 = nc.vector.wait_ge(tsem, 16)

                # w_sum = 2*sum(midw) + 3*sum(tailw)  (dense, exact)
                smid = stat_pool.tile([128, 1], F32, tag="st", name="smid")
                nc.vector.tensor_reduce(smid[:], wbuf[:, 0, :], axis=AX.X,
                                        op=ALU.add)
                stail = stat_pool.tile([128, 1], F32, tag="st", name="stail")
                nc.vector.tensor_reduce(stail[:], wbuf[:, 1, :], axis=AX.X,
                                        op=ALU.add)
                st3 = stat_pool.tile([128, 1], F32, tag="st", name="st3")
                nc.vector.tensor_scalar_mul(st3[:], stail[:], 3.0)
                nc.vector.scalar_tensor_tensor(
                    out=wcol[:], in0=smid[:], scalar=2.0, in1=st3[:],
                    op0=ALU.mult, op1=ALU.add)

                # ===== partner logits: fp8 DoubleRow with 0.7*16*W;
                # rhs granules: [128, 2, NV] at stride (1, 2) per chunk =====
                lp = lp_pool.tile([64, NV], F32, tag="lp", name="lp")
                for k in range(0, KC, 2):
                    rhs8 = _bc(gtm[:], gtm[:, k, 0:1], [[1, 2], [2, NV]])
                    mm = nc.tensor.matmul(lp[:], lhsT=wtl_sb[:, k:k + 2, :],
                                          rhs=rhs8, start=(k == 0),
                                          stop=(k == KC - 2),
                                          perf_mode=mybir.MatmulPerfMode.DoubleRow)
                    if k == 0:
                        add_dep_helper(mm.ins, wtT1.ins, sync=False,
                                       reason="rhs lands with ftab gather")

                # ===== CE numerator/denominator in [class, slot] =====
                zm0 = _bc(zuT[0:C, 0, 0:KU], zuT[0:C, 0, 0:KU],
                          [[0, 2], [1, KU]])
                zt0 = _bc(zuT[0:C, 0, KU:128], zuT[0:C, 0, KU:128],
                          [[0, 3], [1, KU]])
                zm1 = _bc(zuT[0:C, 1, 0:KU], zuT[0:C, 1, 0:KU],
                          [[0, 2], [1, KU]])
                zt1 = _bc(zuT[0:C, 1, KU:128], zuT[0:C, 1, KU:128],
                          [[0, 3], [1, KU]])
                li_ = nc.vector.scalar_tensor_tensor(
                    out=l_sb[:, 0:2 * KU], in0=lp[0:C, 0:2 * KU],
                    scalar=1.0 / 16.0, in1=zm0, op0=ALU.mult, op1=ALU.add)
                add_dep_helper(li_.ins, wtV2.ins, sync=False,
                               reason="zu side lands with zu gather")
                nc.vector.scalar_tensor_tensor(
                    out=l_sb[:, 2 * KU:NV], in0=lp[0:C, 2 * KU:NV],
                    scalar=1.0 / 16.0, in1=zt0, op0=ALU.mult, op1=ALU.add)
                if use_bias:
                    nc.vector.tensor_scalar(
                        out=l_sb[:], in0=l_sb[:],
                        scalar1=biascol_sb[0:C, 0:1], scalar2=None,
                        op0=ALU.add)
                sel = small_pool.tile([C, NV], F32, tag="sel", name="sel")
                # label onehot: fp8 1.0 at even byte 2*class of the
                # 5th 256B column block -> declared chunk index 8
                oh_ap0 = _bc(gtm[0:C, 8, 0:1], gtm[0:C, 8, 0:1],
                             [[2, 2 * KU]])
                se_ = nc.vector.scalar_tensor_tensor(
                    out=sel[:, 0:2 * KU], in0=oh_ap0, scalar=0.7,
                    in1=zm1, op0=ALU.mult, op1=ALU.add)
                add_dep_helper(se_.ins, wt1.ins, sync=False,
                               reason="label onehot lands with ftab gather")
                add_dep_helper(se_.ins, wtV2.ins, sync=False,
                               reason="pred onehot lands with zu gather")
                nc.vector.scalar_tensor_tensor(
                    out=sel[:, 2 * KU:NV],
                    in0=bass.AP(tensor=oh_ap0.tensor,
                                offset=oh_ap0.offset + 4 * KU,
                                ap=[oh_ap0.ap[0], [2, 3 * KU]]),
                    scalar=0.7, in1=zt1, op0=ALU.mult, op1=ALU.add)
                nc.vector.tensor_tensor(out=esel[:, 0, :], in0=l_sb[:],
                                        in1=sel[:], op=ALU.mult)
                nc.scalar.activation(esel[:, 1, :], l_sb[:], AF.Exp)
                red0 = vtp_pool.tile([1, NV], F32, tag="vtp", name="red0")
                nc.tensor.matmul(red0[:], lhsT=ones_bf[0:C, :],
                                 rhs=esel[:, 0, :], start=True, stop=True)
                red1 = pp_pool.tile([1, NV], F32, tag="vtp", name="red1")
                nc.tensor.matmul(red1[:], lhsT=ones_bf[0:C, :],
                                 rhs=esel[:, 1, :], start=True, stop=True)
                nc.vector.tensor_copy(outrow[0:1, 0:NV], red0[:])
                nc.vector.tensor_copy(outrow[0:1, NV:2 * NV], red1[:])
                wps = lp_pool.tile([1, 1], F32, tag="lp", name="wps")
                nc.tensor.matmul(wps[:], lhsT=ones128[:], rhs=wcol[:],
                                 start=True, stop=True)
                nc.vector.tensor_copy(outrow[0:1, 2 * NV:2 * NV + 1], wps[:])
                nc.vector.tensor_copy(
                    outrow[0:1, 2 * NV + 1:2 * NV + 3], cntf[:])
                nc.sync.dma_start(out=out_h[:], in_=outrow[:])

        ppcm.__exit__(None, None, None)

    nc.compile()
    return nc


def make_in_maps(cfg: Cfg, feat, label, W_o, b_o, W, b, gm, gt, idx_m, idx_t):
    """Host-side shard/prep (data movement + casts only). Returns in_maps."""
    n_o, C, CH = cfg.n_o, cfg.c, cfg.chunks
    feat = np.ascontiguousarray(np.asarray(feat, np.float32))
    label = np.asarray(label).astype(np.int64)
    W_o = np.asarray(W_o, np.float32)
    W = np.asarray(W, np.float32)
    b_o = np.asarray(b_o, np.float32)
    b = np.asarray(b, np.float32)
    gm = np.asarray(gm).astype(np.float32)
    gt = np.asarray(gt).astype(np.float32)
    idxs = np.concatenate([np.asarray(idx_m), np.asarray(idx_t)], 0).astype(np.int64)

    use_bias = bool(np.any(b) or np.any(b_o))
    feat_bf = feat.astype(ml_dtypes.bfloat16)

    # full labeled fp8 gather table, rows pre-permuted so the 16-bit
    # granular transpose gather lands DoubleRow pairs correctly:
    # row byte [c16*256 + 2p + b] = feat[(2*c16+b)*128 + p]
    f8 = feat[:n_o].astype(ml_dtypes.float8_e4m3)
    ftab = np.zeros((n_o, cfg.frow), ml_dtypes.float8_e4m3)
    ftab[:, 0:cfg.d] = np.ascontiguousarray(
        f8.reshape(n_o, cfg.d // 256, 2, 128).transpose(0, 1, 3, 2)
        .reshape(n_o, cfg.d))
    ftab[np.arange(n_o), cfg.d + 2 * label[:n_o]] = np.asarray(
        1.0, ml_dtypes.float8_e4m3)

    wtu_f = np.zeros((cfg.d, 128), np.float32)
    wtu_f[:, 0:C] = 0.3 * WSCALE * W.T
    wtu_f[:, 64:64 + C] = WSCALE * W_o.T
    wtu = np.ascontiguousarray(
        wtu_f.reshape(cfg.kc, 128, 128).transpose(1, 0, 2)
        .astype(ml_dtypes.float8_e4m3))
    wtl_f = np.zeros((cfg.d, 64), np.float32)
    wtl_f[:, 0:C] = 0.7 * 16.0 * W.T
    wtl = np.ascontiguousarray(
        wtl_f.reshape(cfg.kc, 128, 64).transpose(1, 0, 2)
        .astype(ml_dtypes.float8_e4m3))

    biascol = np.zeros((128, 2), np.float32)
    biascol[0:C, 0] = b
    biascol[64:64 + C, 1] = b_o
    iotarow_p1 = (np.arange(CH, dtype=np.float32)[None, :] * 128
                  + np.arange(128, dtype=np.float32)[:, None] + 1.0)
    repmat = np.zeros((128, 128), np.float32)
    for l in range(16):
        repmat[l, np.arange(l, 128, 16)] = 1.0  # repmat[l, p]=1 iff p%16==l
    identity = np.eye(128, dtype=np.float32)
    esel_c = np.zeros((128, 4 * 128), np.float32)
    for s1 in range(4):
        for p in range(128):
            esel_c[16 * s1 + (p % 16), 128 * s1 + p] = 1.0

    feat_u8 = (feat[n_o:] * XSCALE).astype(ml_dtypes.float8_e4m3)

    in_maps = []
    for r in range(cfg.cores):
        u0 = cfg.u * r

        def xt_pack(rows):
            a = feat_u8[rows[0]:rows[1]].T  # [d, n]
            n = rows[1] - rows[0]
            a = a.reshape(cfg.kc, 128, n // cfg.rowt, cfg.rowt)
            return np.ascontiguousarray(a.transpose(2, 1, 0, 3))

        idxf = idxs[:, u0:u0 + cfg.u]
        idx_hi = (idxf // 128).astype(np.float32)
        idx_lo = (idxf % 128).astype(np.float32)
        # [128, 5, CH] with consts col layout c*CH + g
        idx_hi = idx_hi.reshape(5, CH, 128).transpose(2, 0, 1)
        idx_lo = idx_lo.reshape(5, CH, 128).transpose(2, 0, 1)
        consts = np.concatenate([
            np.tile(gm, (128, 1)),
            np.tile(gt, (128, 1)),
            iotarow_p1,
            idx_hi.reshape(128, 5 * CH),
            idx_lo.reshape(128, 5 * CH),
            repmat,
            identity,
            esel_c,
        ], axis=1)
        in_maps.append(dict(
            xu=xt_pack((u0, u0 + cfg.u)),
            wtu=wtu,
            wtl=wtl,
            ftab=ftab,
            consts=np.ascontiguousarray(consts),
            biascol=biascol,
        ))
    return in_maps, use_bias


_CACHE = {}


def _get_nc(cfg: Cfg, use_bias: bool):
    key = (cfg.n_o, cfg.n_u, cfg.d, cfg.cores, cfg.rowt, use_bias)
    if key not in _CACHE:
        _CACHE[key] = build_bass(cfg, use_bias)
    return _CACHE[key]


def _install_ntff_shim():
    """This image's antenv lacks axon_hooks; recreate it so trace=True works."""
    import sys
    import types
    try:
        from antenv.axon_hooks import get_axon_ntff_profile_hook  # noqa: F401
        return
    except ImportError:
        pass
    try:
        import antenv
        from trn_agent_boot.trn_boot import _ntff_profile_via_ctypes
        h = _ntff_profile_via_ctypes("/opt/axon/libaxon_pjrt.so")
        mod = types.ModuleType("antenv.axon_hooks")
        mod.get_axon_ntff_profile_hook = lambda: h
        mod.set_axon_ntff_profile_hook = lambda hook: None
        sys.modules["antenv.axon_hooks"] = mod
        antenv.axon_hooks = mod
    except Exception:
        pass


def kernel(feat, label, W_o, b_o, W, b, group_mid_mask, group_tail_mask,
           idx_m, idx_t, _trace=False):
    if _trace:
        _install_ntff_shim()
    n_u = int(np.asarray(idx_m).shape[1])
    n_o = int(np.asarray(feat).shape[0]) - n_u
    cfg = Cfg(n_o=n_o, n_u=n_u, d=int(np.asarray(feat).shape[1]))
    in_maps, use_bias = make_in_maps(cfg, feat, label, W_o, b_o, W, b,
                                     group_mid_mask, group_tail_mask,
                                     idx_m, idx_t)
    nc = _get_nc(cfg, use_bias)
    res = run_bass_kernel_spmd(nc, in_maps, core_ids=list(range(cfg.cores)),
                               trace=_trace)
    # host finish: ln() over valid slots + partial sums (the "unshard")
    ce_sum = 0.0
    w_sum = 0.0
    for r in range(cfg.cores):
        row = np.asarray(res.results[r]["out"]).reshape(-1)
        red0, red1 = row[0:NV], row[NV:2 * NV]
        w_sum += float(row[2 * NV])
        nm = int(row[2 * NV + 1])
        nt = int(row[2 * NV + 2])
        for c in range(5):
            n = min(nm, KU) if c < 2 else min(nt, KU)
            if n <= 0:
                continue
            s = slice(c * KU, c * KU + n)
            ce_sum += float((np.log(red1[s]) - red0[s]).sum())
    out = np.float32(ce_sum / max(w_sum, 1.0))
    if _trace:
        return out, res
    return out


# revision 38
# speedup vs baseline: 1.0388x; 1.0388x over previous
"""Trainium2 Bass kernel for nn_BalanceLabelAugmentation2 (topk_masking).

Math (reference, restructured):
  Z   = feat @ W.T            [N, 51]   (matmul is linear over the mixup!)
  lo  = feat_u @ W_o.T + b_o  [N_u, 51] -> pred=argmax, score=max softmax
  midw_i  = gm[pred_i] & (score_i > 0.5);  tailw_i = gt[pred_i] & (score_i > 0.3)
  For pair (copy c, unlabeled row i) with partner j = idx_c[i]:
    l    = 0.7*Z_o[j] + 0.3*Z_u[i] + b
    ce   = logsumexp(l) - 0.7*l[label_j] - 0.3*l[pred_i]
  out = sum(ce*w) / max(sum w, 1)

NO-COLLECTIVE design (v6).  Every core receives a full local copy of the
labeled FEATURES (+0.7*onehot(label)) as a DRAM gather table; zero
collectives.  The device ships per-slot [sum(l*sel) | sum(exp l)] rows
plus the compaction counts and the dense weight sum; the host (which
already sums the per-core partials) finishes with ln() on the <=320
valid slots per core.

Per core:
  Phase B   fp8 DoubleRow matmul of the unlabeled shard with
            [0.3*W | W_o] heads (feat*8 and weights*8 host scales, PSUM
            descale 1/64), one [115,128] PE transpose per 128-row chunk,
            and PER-TILE mask passes that overlap the next tile's
            matmul.  zu rows (256 bf16 elems) carry [0.3*Zu | j_hi | ...
            | 0.3*OHpred | j_lo] where j_hi/j_lo split the partner idx
            into bf16-exact 7-bit halves (partitions 64:69, chunks 0/1).
  Compact   TWO sparse_gathers (mid row-ids, tail row-ids) -> counts;
            handles arbitrary (even overlapping) group masks exactly.
  Gathers   zu idx staged via a replication MATMUL; transpose-mode zu
            gather (slots 0:64 mid, 64:128 tail); partner j rebuilt from
            the gathered hi/lo cols and staged DMA-FREE into the 16-lane
            idx layout (transpose + 4 lane-select matmuls + replication
            matmul); transpose-mode ftab gather (384 idx, 320 live).
  CE        [class, slot]: l = 8-chunk 0.7*W matmul + zu broadcast (+b);
            sel = 0.7*OHlab + 0.3*OHpred; per-slot class sums via two
            ones-matmuls -> shipped to host.
"""

import numpy as np
import ml_dtypes

import concourse.bass as bass
import concourse.tile as tile
from concourse import bacc, mybir
from concourse.bass_utils import run_bass_kernel_spmd
from concourse.tile_rust import add_dep_helper

F32 = mybir.dt.float32
BF16 = mybir.dt.bfloat16
FP8 = mybir.dt.float8e4
XSCALE = 8.0   # host feat scale (avoids e4m3 subnormals)
WSCALE = 8.0   # host weight scale; PSUM descale = 1/(XSCALE*WSCALE)
I16 = mybir.dt.int16
I32 = mybir.dt.int32
U32 = mybir.dt.uint32
AF = mybir.ActivationFunctionType
ALU = mybir.AluOpType
AX = mybir.AxisListType

KU = 64                # per-group survivor capacity (worst observed ~27)
NV = 5 * KU            # live ftab slots (320); slot k = 64c + s
NT = 6 * KU            # ftab gather idx count incl pad block (%128==0)
NW = NT // 16          # staged idx words (24)
NOUT = 2 * NV + 4      # [red0 | red1 | wsum, nm, nt, pad]


class Cfg:
    def __init__(self, n_o=16384, n_u=16384, d=1024, cores=8, rowt=512):
        self.n_o, self.n_u, self.d, self.cores, self.rowt = n_o, n_u, d, cores, rowt
        self.c = 51
        self.u = n_u // cores          # unlabeled rows per core
        self.kc = d // 128             # contraction chunks
        self.unl_tiles = self.u // rowt
        self.cpt = rowt // 128         # 128-row chunks per tile
        self.chunks = self.u // 128    # unlabeled 128-row chunks
        self.frow = d + 256            # fp8 ftab row bytes (feat + onehot)
        self.zrow = 256                # zu row bf16 elems (512B)
        assert self.u % rowt == 0 and d % 128 == 0
        assert self.chunks <= 16


def _bc(tile_ap, offset_ap, pattern):
    """AP on tile_ap's tensor at offset_ap's offset with a custom free pattern."""
    return bass.AP(tensor=tile_ap.tensor, offset=offset_ap.offset,
                   ap=[tile_ap.ap[0]] + pattern)


def build_bass(cfg: Cfg, use_bias: bool):
    C, FROW, ZROW, KC, ROWT = cfg.c, cfg.frow, cfg.zrow, cfg.kc, cfg.rowt
    WTC = 128  # [0.3W | pad | W_o | pad], fp8 dual-weights want even cols
    CH = cfg.chunks
    nc = bacc.Bacc("TRN2", target_bir_lowering=False, debug=False,
                   num_devices=cfg.cores)

    xu_h = nc.dram_tensor("xu", [cfg.unl_tiles, 128, KC, ROWT], FP8,
                          kind="ExternalInput")
    wtu_h = nc.dram_tensor("wtu", [128, KC, WTC], FP8, kind="ExternalInput")
    wtl_h = nc.dram_tensor("wtl", [128, KC, 64], FP8, kind="ExternalInput")
    ftab_h = nc.dram_tensor("ftab", [cfg.n_o, FROW], FP8,
                            kind="ExternalInput")
    # consts cols: gm[0:C], gt[C:2C], iotarow_p1[2C : 2C+CH],
    # idx_hi [.. 5CH], idx_lo [.. 5CH], repmat [.. 128], identity [.. 128],
    # fused lane-select E'_s1[s, p] = (s == 16*s1 + p%16)  [.. 4*128]
    O_GM, O_GT = 0, C
    O_RP1 = 2 * C
    O_HI = O_RP1 + CH
    O_LO = O_HI + 5 * CH
    O_REP = O_LO + 5 * CH
    O_ID = O_REP + 128
    O_ES = O_ID + 128
    NCONST = O_ES + 4 * 128
    consts_h = nc.dram_tensor("consts", [128, NCONST], F32, kind="ExternalInput")
    biascol_h = nc.dram_tensor("biascol", [WTC, 2], F32, kind="ExternalInput")
    out_h = nc.dram_tensor("out", [1, NOUT], F32, kind="ExternalOutput")
    zud_h = nc.dram_tensor("zud", [16384 // 8, 256], BF16)
    offs_h = nc.dram_tensor("offs", [128, 1], I32)

    with tile.TileContext(nc) as tc:
        ppcm = tc.tile_pool(name="persist", bufs=1)
        pp_ = ppcm.__enter__()

        def P(shape, dtype, name):
            return pp_.tile(shape, dtype, name=name, tag=name)

        wtu_sb = P([128, KC, WTC], FP8, "wtu_sb")
        wtl_sb = P([128, KC, 64], FP8, "wtl_sb")
        consts_sb = P([128, NCONST], F32, "consts_sb")
        gm_r = consts_sb[:, O_GM:O_GM + C]
        gt_r = consts_sb[:, O_GT:O_GT + C]
        iotarow_p1 = consts_sb[:, O_RP1:O_RP1 + CH]
        repmat = consts_sb[0:16, O_REP:O_REP + 128]
        ident = consts_sb[:, O_ID:O_ID + 128]
        if use_bias:
            biascol_sb = P([WTC, 2], F32, "biascol_sb")
            nc.sync.dma_start(out=biascol_sb[:], in_=biascol_h[:])
        ones128 = P([128, 1], F32, "ones128")
        nc.vector.memset(ones128[:], 1.0)
        ones_bf = P([128, 1], BF16, "ones_bf")
        nc.vector.memset(ones_bf[:], 1.0)
        ident_bf = P([128, 128], BF16, "ident_bf")
        ibf_wr = None  # cast from consts identity once it lands

        zu_all = P([128, CH, ZROW], BF16, "zu_all")
        lo_all = P([128, CH, C], F32, "lo_all")
        wbuf = P([128, 2, CH], F32, "wbuf")
        vpack = P([128, 2, CH], F32, "vpack")
        vt_sb = P([16, 2, 128], F32, "vt_sb")
        sgMT = P([16, 8], F32, "sgMT")           # [mid | tail] rowids
        cnt8 = P([1, 8], U32, "cnt8")
        cntf = P([1, 2], F32, "cntf")
        stgBi = P([16, 8], I32, "stgBi")
        offs_col = P([128, 1], I32, "offs_col")
        zrows = P([128, ZROW], BF16, "zrows")
        jf128 = P([128, KU], F32, "jf128")       # j rebuilt at parts 64:69
        jT_sb = P([KU, 8], F32, "jT_sb")         # transposed j [s, c]
        stg16 = P([16, NW], F32, "stg16")        # lane-word wrapped j
        nc.vector.memset(stg16[:], 0.0)          # pad words 20:24 stay 0
        idx_sbA = P([128, NW], I16, "idx_sbA")
        nc.vector.memset(idx_sbA[:, 20:NW], 0)
        gtm = P([128, KC + 2, NT], FP8, "gtm")   # transposed fp8 ftab
        zuT = P([128, 2, 128], F32, "zuT")        # transposed zu rows
        l_sb = P([C, NV], F32, "l_sb")
        esel = P([C, 2, NV], BF16, "esel")        # [l*sel | e]
        wcol = P([128, 1], F32, "wcol")
        outrow = P([1, NOUT], F32, "outrow")
        nc.vector.memset(outrow[0:1, NOUT - 1:NOUT], 0.0)

        tsem = nc.alloc_semaphore("tsem")
        zsem = nc.alloc_semaphore("zsem")

        if True:
            with (
                tc.tile_pool(name="xt", bufs=cfg.unl_tiles) as xt_pool,
                tc.tile_pool(name="ztp", bufs=2, space="PSUM") as zt_pool,
                tc.tile_pool(name="zts", bufs=4) as zts_pool,
                tc.tile_pool(name="trp", bufs=3, space="PSUM") as tr_pool,
                tc.tile_pool(name="vtpp", bufs=1, space="PSUM") as vtp_pool,
                tc.tile_pool(name="ppp", bufs=1, space="PSUM") as pp_pool,
                tc.tile_pool(name="lpp", bufs=1, space="PSUM") as lp_pool,
                tc.tile_pool(name="small", bufs=8) as small_pool,
                tc.tile_pool(name="stat", bufs=24) as stat_pool,
            ):
                def xtile_load(t):
                    xt = xt_pool.tile([128, KC, ROWT], FP8, name="xt",
                                      tag="xtu")
                    nc.scalar.dma_start(
                        out=xt[:],
                        in_=bass.AP(tensor=xu_h, offset=t * 128 * KC * ROWT,
                                    ap=[[KC * ROWT, 128], [ROWT, KC],
                                        [1, ROWT]]))
                    return xt

                nc.sync.dma_start(out=wtu_sb[:], in_=wtu_h[:])
                xus = [xtile_load(t) for t in range(cfg.unl_tiles)]
                nc.sync.dma_start(out=wtl_sb[:], in_=wtl_h[:])
                nc.sync.dma_start(out=consts_sb[:], in_=consts_h[:])
                nc.vector.tensor_copy(ident_bf[:], ident)
                # partner idx halves into the zu rows (bf16-exact 7-bit):
                # mid copies at row elems 64:66 / 192:194, tail copies at
                # 96:99 / 224:227 (32-aligned partition bases post-gather)
                for eo, co, ncp in ((64, 0, 2), (96, 2 * CH, 3)):
                    nc.vector.tensor_copy(
                        _bc(zu_all[:], zu_all[:, 0:1, eo:eo + 1],
                            [[ZROW, CH], [1, ncp]]),
                        _bc(consts_sb[:], consts_sb[:, O_HI + co:O_HI + co + 1],
                            [[1, CH], [CH, ncp]]))
                    nc.vector.tensor_copy(
                        _bc(zu_all[:], zu_all[:, 0:1, 128 + eo:128 + eo + 1],
                            [[ZROW, CH], [1, ncp]]),
                        _bc(consts_sb[:], consts_sb[:, O_LO + co:O_LO + co + 1],
                            [[1, CH], [CH, ncp]]))

                def matmul_tile_fp8(xt):
                    zt = zt_pool.tile([WTC, ROWT], F32, tag="zt", name="zt")
                    for k in range(0, KC, 2):
                        nc.tensor.matmul(
                            zt[:], lhsT=wtu_sb[:, k:k + 2, :],
                            rhs=xt[:, k:k + 2, :], start=(k == 0),
                            stop=(k == KC - 2),
                            perf_mode=mybir.MatmulPerfMode.DoubleRow)
                    return zt

                # ========== Phase B + per-tile masks (overlapped) ==========
                for t in range(cfg.unl_tiles):
                    zt = matmul_tile_fp8(xus[t])
                    zq = zts_pool.tile([WTC, ROWT], F32, tag="zq", name="zq")
                    nc.scalar.activation(
                        zq[:], zt[:], AF.Copy,
                        bias=(biascol_sb[:, 1:2] if use_bias else 0.0),
                        scale=1.0 / (XSCALE * WSCALE))
                    for q in range(cfg.cpt):
                        g = t * cfg.cpt + q
                        qs = slice(q * 128, (q + 1) * 128)
                        tr = tr_pool.tile([128, 64 + C], F32, tag="tr",
                                          name="tr")
                        nc.tensor.transpose(tr[:], zq[0:64 + C, qs],
                                            ident[0:64 + C, 0:64 + C])
                        nc.scalar.copy(zu_all[:, g, 0:C], tr[:, 0:C])
                        nc.scalar.copy(lo_all[:, g, :], tr[:, 64:64 + C])

                    # masks for this tile's chunks [128, cpt, C]
                    sl = slice(t * cfg.cpt, (t + 1) * cfg.cpt)
                    lo_t = lo_all[:, sl, :]
                    TP = cfg.cpt
                    mx = stat_pool.tile([128, TP], F32, tag="st", name="mx")
                    nc.vector.tensor_reduce(mx[:], lo_t, axis=AX.X,
                                            op=ALU.max)
                    ew = small_pool.tile([128, TP, C], F32, tag="ew",
                                         name="ew")
                    nc.scalar.activation(ew[:], lo_t, AF.Exp)
                    ssum = stat_pool.tile([128, TP], F32, tag="st",
                                          name="ssum")
                    nc.vector.tensor_reduce(ssum[:], ew[:], axis=AX.X,
                                            op=ALU.add)
                    em = stat_pool.tile([128, TP], F32, tag="st", name="em")
                    nc.scalar.activation(em[:], mx[:], AF.Exp)
                    mx_bc = _bc(mx[:], mx[:], [[1, TP], [0, C]])
                    oh = small_pool.tile([128, TP, C], F32, tag="oh",
                                         name="oh")
                    nc.vector.tensor_tensor(out=oh[:], in0=lo_t, in1=mx_bc,
                                            op=ALU.is_equal)
                    # 0.3*onehot(pred) into the zu rows (bf16 cast on write)
                    nc.vector.tensor_scalar_mul(
                        zu_all[:, sl, 128:128 + C], oh[:], 0.3)
                    gm_bc = _bc(consts_sb[:], gm_r, [[0, TP], [1, C]])
                    gt_bc = _bc(consts_sb[:], gt_r, [[0, TP], [1, C]])
                    jm = small_pool.tile([128, TP, C], F32, tag="ew",
                                         name="jm")
                    nc.vector.tensor_tensor(out=jm[:], in0=oh[:], in1=gm_bc,
                                            op=ALU.mult)
                    gvm = stat_pool.tile([128, TP], F32, tag="st", name="gvm")
                    nc.vector.tensor_reduce(gvm[:], jm[:], axis=AX.X,
                                            op=ALU.add)
                    jt = small_pool.tile([128, TP, C], F32, tag="oh",
                                         name="jt")
                    nc.vector.tensor_tensor(out=jt[:], in0=oh[:], in1=gt_bc,
                                            op=ALU.mult)
                    gvt = stat_pool.tile([128, TP], F32, tag="st", name="gvt")
                    nc.vector.tensor_reduce(gvt[:], jt[:], axis=AX.X,
                                            op=ALU.add)
                    # score>thr  <=>  thr*sum(e) < e^max
                    tm = stat_pool.tile([128, TP], F32, tag="st", name="tm")
                    nc.vector.scalar_tensor_tensor(
                        out=tm[:], in0=ssum[:], scalar=0.5, in1=em[:],
                        op0=ALU.mult, op1=ALU.is_lt)
                    nc.vector.tensor_tensor(out=wbuf[:, 0, sl], in0=tm[:],
                                            in1=gvm[:], op=ALU.mult)
                    tt = stat_pool.tile([128, TP], F32, tag="st", name="tt")
                    nc.vector.scalar_tensor_tensor(
                        out=tt[:], in0=ssum[:], scalar=0.3, in1=em[:],
                        op0=ALU.mult, op1=ALU.is_lt)
                    nc.vector.tensor_tensor(out=wbuf[:, 1, sl], in0=tt[:],
                                            in1=gvt[:], op=ALU.mult)
                    # v-list cols: v = (rowid+1)*w - 1 for mid and tail
                    for li in range(2):
                        nc.vector.tensor_tensor(
                            out=vpack[:, li, sl], in0=wbuf[:, li, sl],
                            in1=iotarow_p1[:, sl], op=ALU.mult)
                        nc.vector.tensor_scalar_add(vpack[:, li, sl],
                                                    vpack[:, li, sl], -1.0)
                    # zu rows of this tile -> DRAM (overlaps next tile)
                    zwr = nc.sync.dma_start(
                        out=bass.AP(tensor=zud_h,
                                    offset=t * ROWT * ZROW,
                                    ap=[[ZROW, 128], [128 * ZROW, TP],
                                        [1, ZROW]]),
                        in_=zu_all[:, sl, :])
                    if t == cfg.unl_tiles - 1:
                        zu_wr = zwr

                # ===== compaction: 2 sparse_gathers (mid/tail rowids) =====
                with tc.high_priority(offset=400):
                    for li in range(2):
                        pool = vtp_pool if li == 0 else pp_pool
                        vt_ps = pool.tile([16, 128], F32, tag="vtp",
                                          name="vt_ps")
                        nc.tensor.transpose(vt_ps[:], vpack[:, li, :],
                                            ident[:])
                        nc.vector.tensor_copy(vt_sb[:, li, :], vt_ps[:])
                    for li in range(2):
                        nc.gpsimd.sparse_gather(
                            out=sgMT[0:16, li * 4:(li + 1) * 4],
                            in_=vt_sb[:, li, :],
                            num_found=cnt8[0:1, li:li + 1])
                    nc.vector.tensor_copy(cntf[:], cnt8[0:1, 0:2])

                    # zu row gather via the DGE indirect path (no
                    # swdge ucode lib -> the sparse->dma lib switch can
                    # overlap this).  Offsets staged lane-major through
                    # DRAM: slot p = lane*W + word, i.e. compacted index
                    # perm(p) = (p%W)*16 + p//W per list block.
                    nc.vector.tensor_scalar(
                        out=stgBi[:], in0=sgMT[:],
                        scalar1=0.0, scalar2=float(cfg.u - 1),
                        op0=ALU.max, op1=ALU.min)
                    nc.sync.dma_start(
                        out=bass.AP(tensor=offs_h, offset=0,
                                    ap=[[4, 16], [1, 4]]),
                        in_=stgBi[:, 0:4])
                    nc.sync.dma_start(
                        out=bass.AP(tensor=offs_h, offset=64,
                                    ap=[[4, 16], [1, 4]]),
                        in_=stgBi[:, 4:8])
                    nc.sync.dma_start(
                        out=offs_col[:],
                        in_=bass.AP(tensor=offs_h, offset=0,
                                    ap=[[1, 128], [1, 1]]))
                    gB = nc.gpsimd.indirect_dma_start(
                        out=zrows[:], out_offset=None,
                        in_=bass.AP(tensor=zud_h, offset=0,
                                    ap=[[ZROW, cfg.u], [1, ZROW]]),
                        in_offset=bass.IndirectOffsetOnAxis(
                            ap=offs_col[:, 0:1], axis=0))
                    gB.then_inc(zsem, 16)
                    add_dep_helper(gB.ins, zu_wr.ins, sync=True,
                                   reason="zu gather reads zu_dram")

                    wtV2 = nc.vector.wait_ge(zsem, 16)
                    wtT2 = nc.tensor.wait_ge(zsem, 16)
                    # [slot, elem] -> [elem-partition, slot] via PE
                    for ch in range(2):
                        ztp = tr_pool.tile([128, 128], BF16, tag="tr",
                                           name="ztp")
                        tmm = nc.tensor.transpose(
                            ztp[:], zrows[:, 128 * ch:128 * ch + 128],
                            ident_bf[:])
                        if ch == 0:
                            add_dep_helper(tmm.ins, wtT2.ins, sync=False,
                                           reason="zu rows landed")
                        nc.vector.tensor_copy(zuT[:, ch, :], ztp[:])

                    # rebuild partner j = 128*hi + lo (mid at parts 64:66
                    # over slots 0:64, tail at parts 96:99 over 64:128)
                    jb1 = nc.vector.scalar_tensor_tensor(
                        out=jf128[64:66, :], in0=zuT[64:66, 0, 0:KU],
                        scalar=128.0, in1=zuT[64:66, 1, 0:KU],
                        op0=ALU.mult, op1=ALU.add)
                    add_dep_helper(jb1.ins, wtV2.ins, sync=False,
                                   reason="j cols land with zu gather")
                    nc.vector.scalar_tensor_tensor(
                        out=jf128[32:35, :], in0=zuT[96:99, 0, KU:128],
                        scalar=128.0, in1=zuT[96:99, 1, KU:128],
                        op0=ALU.mult, op1=ALU.add)
                    # DMA-free staging into the gather idx layout
                    # (slot k=64c+s -> lane s%16, word 4c+s//16):
                    # transpose j -> [s, c], 4 lane-select matmuls
                    # staged[l, 4c+s1] = jT[16*s1+l, c], one replication
                    # matmul to 128 partitions, clamp+cast to i16.
                    jt_ps = tr_pool.tile([KU, 8], F32, tag="tr",
                                         name="jt_ps")
                    nc.tensor.transpose(jt_ps[0:KU, 0:2], jf128[64:66, :],
                                        ident[64:66, 64:66])
                    nc.tensor.transpose(jt_ps[0:KU, 2:5], jf128[32:35, :],
                                        ident[32:35, 32:35])
                    nc.vector.tensor_copy(jT_sb[:, 0:5], jt_ps[0:KU, 0:5])
                    # fused lane-select + 8x sub-core replication:
                    # idx_sbA[p, 4c+s1] = jT[16*s1 + p%16, c]
                    for s1 in range(4):
                        es_ps = vtp_pool.tile([128, 8], F32, tag="vtp",
                                              name="es_ps")
                        nc.tensor.matmul(
                            es_ps[0:128, 0:5],
                            lhsT=consts_sb[0:KU, O_ES + 128 * s1:
                                           O_ES + 128 * s1 + 128],
                            rhs=jT_sb[:, 0:5], start=True, stop=True)
                        nc.vector.tensor_scalar(
                            out=_bc(idx_sbA[:], idx_sbA[:, s1:s1 + 1],
                                    [[4, 5]]),
                            in0=es_ps[0:128, 0:5],
                            scalar1=0.0, scalar2=float(cfg.n_o - 1),
                            op0=ALU.max, op1=ALU.min)

                    gA = nc.gpsimd.dma_gather(
                        out_ap=gtm[:], in_ap=ftab_h[:],
                        idxs_ap=idx_sbA[:],
                        num_idxs=NT, num_idxs_reg=NT, elem_size=FROW,
                        transpose=True)
                    gA.then_inc(tsem, 16)

                    wtT1 = nc.tensor.wait_ge(tsem, 16)
                    wt1 = nc.vector.wait_ge(tsem, 16)

                # w_sum = 2*sum(midw) + 3*sum(tailw)  (dense, exact)
                smid = stat_pool.tile([128, 1], F32, tag="st", name="smid")
                nc.vector.tensor_reduce(smid[:], wbuf[:, 0, :], axis=AX.X,
                                        op=ALU.add)
                stail = stat_pool.tile([128, 1], F32, tag="st", name="stail")
                nc.vector.tensor_reduce(stail[:], wbuf[:, 1, :], axis=AX.X,
                                        op=ALU.add)
                st3 = stat_pool.tile([128, 1], F32, tag="st", name="st3")
                nc.vector.tensor_scalar_mul(st3[:], stail[:], 3.0)
                nc.vector.scalar_tensor_tensor(
                    out=wcol[:], in0=smid[:], scalar=2.0, in1=st3[:],
                    op0=ALU.mult, op1=ALU.add)

                # ===== partner logits: fp8 DoubleRow with 0.7*16*W;
                # rhs granules: [128, 2, NV] at stride (1, 2) per chunk =====
                lp = lp_pool.tile([64, NV], F32, tag="lp", name="lp")
                for k in range(0, KC, 2):
                    rhs8 = _bc(gtm[:], gtm[:, k, 0:1], [[1, 2], [2, NV]])
                    mm = nc.tensor.matmul(lp[:], lhsT=wtl_sb[:, k:k + 2, :],
                                          rhs=rhs8, start=(k == 0),
                                          stop=(k == KC - 2),
                                          perf_mode=mybir.MatmulPerfMode.DoubleRow)
                    if k == 0:
                        add_dep_helper(mm.ins, wtT1.ins, sync=False,
                                       reason="rhs lands with ftab gather")

                # ===== CE numerator/denominator in [class, slot] =====
                zm0 = _bc(zuT[0:C, 0, 0:KU], zuT[0:C, 0, 0:KU],
                          [[0, 2], [1, KU]])
                zt0 = _bc(zuT[0:C, 0, KU:128], zuT[0:C, 0, KU:128],
                          [[0, 3], [1, KU]])
                zm1 = _bc(zuT[0:C, 1, 0:KU], zuT[0:C, 1, 0:KU],
                          [[0, 2], [1, KU]])
                zt1 = _bc(zuT[0:C, 1, KU:128], zuT[0:C, 1, KU:128],
                          [[0, 3], [1, KU]])
                li_ = nc.vector.scalar_tensor_tensor(
                    out=l_sb[:, 0:2 * KU], in0=lp[0:C, 0:2 * KU],
                    scalar=1.0 / 16.0, in1=zm0, op0=ALU.mult, op1=ALU.add)
                add_dep_helper(li_.ins, wtV2.ins, sync=False,
                               reason="zu side lands with zu gather")
                nc.vector.scalar_tensor_tensor(
                    out=l_sb[:, 2 * KU:NV], in0=lp[0:C, 2 * KU:NV],
                    scalar=1.0 / 16.0, in1=zt0, op0=ALU.mult, op1=ALU.add)
                if use_bias:
                    nc.vector.tensor_scalar(
                        out=l_sb[:], in0=l_sb[:],
                        scalar1=biascol_sb[0:C, 0:1], scalar2=None,
                        op0=ALU.add)
                sel = small_pool.tile([C, NV], F32, tag="sel", name="sel")
                # label onehot: fp8 1.0 at even byte 2*class of the
                # 5th 256B column block -> declared chunk index 8
                oh_ap0 = _bc(gtm[0:C, 8, 0:1], gtm[0:C, 8, 0:1],
                             [[2, 2 * KU]])
                se_ = nc.vector.scalar_tensor_tensor(
                    out=sel[:, 0:2 * KU], in0=oh_ap0, scalar=0.7,
                    in1=zm1, op0=ALU.mult, op1=ALU.add)
                add_dep_helper(se_.ins, wt1.ins, sync=False,
                               reason="label onehot lands with ftab gather")
                add_dep_helper(se_.ins, wtV2.ins, sync=False,
                               reason="pred onehot lands with zu gather")
                nc.vector.scalar_tensor_tensor(
                    out=sel[:, 2 * KU:NV],
                    in0=bass.AP(tensor=oh_ap0.tensor,
                                offset=oh_ap0.offset + 4 * KU,
                                ap=[oh_ap0.ap[0], [2, 3 * KU]]),
                    scalar=0.7, in1=zt1, op0=ALU.mult, op1=ALU.add)
                nc.vector.tensor_tensor(out=esel[:, 0, :], in0=l_sb[:],
                                        in1=sel[:], op=ALU.mult)
                nc.scalar.activation(esel[:, 1, :], l_sb[:], AF.Exp)
                red0 = vtp_pool.tile([1, NV], F32, tag="vtp", name="red0")
                nc.tensor.matmul(red0[:], lhsT=ones_bf[0:C, :],
                                 rhs=esel[:, 0, :], start=True, stop=True)
                red1 = pp_pool.tile([1, NV], F32, tag="vtp", name="red1")
                nc.tensor.matmul(red1[:], lhsT=ones_bf[0:C, :],
                                 rhs=esel[:, 1, :], start=True, stop=True)
                nc.vector.tensor_copy(outrow[0:1, 0:NV], red0[:])
                nc.vector.tensor_copy(outrow[0:1, NV:2 * NV], red1[:])
                wps = lp_pool.tile([1, 1], F32, tag="lp", name="wps")
                nc.tensor.matmul(wps[:], lhsT=ones128[:], rhs=wcol[:],
                                 start=True, stop=True)
                nc.vector.tensor_copy(outrow[0:1, 2 * NV:2 * NV + 1], wps[:])
                nc.vector.tensor_copy(
                    outrow[0:1, 2 * NV + 1:2 * NV + 3], cntf[:])
                nc.sync.dma_start(out=out_h[:], in_=outrow[:])

        ppcm.__exit__(None, None, None)

    nc.compile()
    return nc


def make_in_maps(cfg: Cfg, feat, label, W_o, b_o, W, b, gm, gt, idx_m, idx_t):
    """Host-side shard/prep (data movement + casts only). Returns in_maps."""
    n_o, C, CH = cfg.n_o, cfg.c, cfg.chunks
    feat = np.ascontiguousarray(np.asarray(feat, np.float32))
    label = np.asarray(label).astype(np.int64)
    W_o = np.asarray(W_o, np.float32)
    W = np.asarray(W, np.float32)
    b_o = np.asarray(b_o, np.float32)
    b = np.asarray(b, np.float32)
    gm = np.asarray(gm).astype(np.float32)
    gt = np.asarray(gt).astype(np.float32)
    idxs = np.concatenate([np.asarray(idx_m), np.asarray(idx_t)], 0).astype(np.int64)

    use_bias = bool(np.any(b) or np.any(b_o))
    feat_bf = feat.astype(ml_dtypes.bfloat16)

    # full labeled fp8 gather table, rows pre-permuted so the 16-bit
    # granular transpose gather lands DoubleRow pairs correctly:
    # row byte [c16*256 + 2p + b] = feat[(2*c16+b)*128 + p]
    f8 = feat[:n_o].astype(ml_dtypes.float8_e4m3)
    ftab = np.zeros((n_o, cfg.frow), ml_dtypes.float8_e4m3)
    ftab[:, 0:cfg.d] = np.ascontiguousarray(
        f8.reshape(n_o, cfg.d // 256, 2, 128).transpose(0, 1, 3, 2)
        .reshape(n_o, cfg.d))
    ftab[np.arange(n_o), cfg.d + 2 * label[:n_o]] = np.asarray(
        1.0, ml_dtypes.float8_e4m3)

    wtu_f = np.zeros((cfg.d, 128), np.float32)
    wtu_f[:, 0:C] = 0.3 * WSCALE * W.T
    wtu_f[:, 64:64 + C] = WSCALE * W_o.T
    wtu = np.ascontiguousarray(
        wtu_f.reshape(cfg.kc, 128, 128).transpose(1, 0, 2)
        .astype(ml_dtypes.float8_e4m3))
    wtl_f = np.zeros((cfg.d, 64), np.float32)
    wtl_f[:, 0:C] = 0.7 * 16.0 * W.T
    wtl = np.ascontiguousarray(
        wtl_f.reshape(cfg.kc, 128, 64).transpose(1, 0, 2)
        .astype(ml_dtypes.float8_e4m3))

    biascol = np.zeros((128, 2), np.float32)
    biascol[0:C, 0] = b
    biascol[64:64 + C, 1] = b_o
    iotarow_p1 = (np.arange(CH, dtype=np.float32)[None, :] * 128
                  + np.arange(128, dtype=np.float32)[:, None] + 1.0)
    repmat = np.zeros((128, 128), np.float32)
    for l in range(16):
        repmat[l, np.arange(l, 128, 16)] = 1.0  # repmat[l, p]=1 iff p%16==l
    identity = np.eye(128, dtype=np.float32)
    esel_c = np.zeros((128, 4 * 128), np.float32)
    for s1 in range(4):
        for p in range(128):
            esel_c[16 * s1 + (p % 16), 128 * s1 + p] = 1.0

    feat_u8 = (feat[n_o:] * XSCALE).astype(ml_dtypes.float8_e4m3)

    in_maps = []
    for r in range(cfg.cores):
        u0 = cfg.u * r

        def xt_pack(rows):
            a = feat_u8[rows[0]:rows[1]].T  # [d, n]
            n = rows[1] - rows[0]
            a = a.reshape(cfg.kc, 128, n // cfg.rowt, cfg.rowt)
            return np.ascontiguousarray(a.transpose(2, 1, 0, 3))

        idxf = idxs[:, u0:u0 + cfg.u]
        idx_hi = (idxf // 128).astype(np.float32)
        idx_lo = (idxf % 128).astype(np.float32)
        # [128, 5, CH] with consts col layout c*CH + g
        idx_hi = idx_hi.reshape(5, CH, 128).transpose(2, 0, 1)
        idx_lo = idx_lo.reshape(5, CH, 128).transpose(2, 0, 1)
        consts = np.concatenate([
            np.tile(gm, (128, 1)),
            np.tile(gt, (128, 1)),
            iotarow_p1,
            idx_hi.reshape(128, 5 * CH),
            idx_lo.reshape(128, 5 * CH),
            repmat,
            identity,
            esel_c,
        ], axis=1)
        in_maps.append(dict(
            xu=xt_pack((u0, u0 + cfg.u)),
            wtu=wtu,
            wtl=wtl,
            ftab=ftab,
            consts=np.ascontiguousarray(consts),
            biascol=biascol,
        ))
    return in_maps, use_bias


_CACHE = {}


def _get_nc(cfg: Cfg, use_bias: bool):
    key = (cfg.n_o, cfg.n_u, cfg.d, cfg.cores, cfg.rowt, use_bias)
    if key not in _CACHE:
        _CACHE[key] = build_bass(cfg, use_bias)
    return _CACHE[key]


def _install_ntff_shim():
    """This image's antenv lacks axon_hooks; recreate it so trace=True works."""
    import sys
    import types
    try:
        from antenv.axon_hooks import get_axon_ntff_profile_hook  # noqa: F401
        return
    except ImportError:
        pass
    try:
        import antenv
        from trn_agent_boot.trn_boot import _ntff_profile_via_ctypes
        h = _ntff_profile_via_ctypes("/opt/axon/libaxon_pjrt.so")
        mod = types.ModuleType("antenv.axon_hooks")
        mod.get_axon_ntff_profile_hook = lambda: h
        mod.set_axon_ntff_profile_hook = lambda hook: None
        sys.modules["antenv.axon_hooks"] = mod
        antenv.axon_hooks = mod
    except Exception:
        pass


def kernel(feat, label, W_o, b_o, W, b, group_mid_mask, group_tail_mask,
           idx_m, idx_t, _trace=False):
    if _trace:
        _install_ntff_shim()
    n_u = int(np.asarray(idx_m).shape[1])
    n_o = int(np.asarray(feat).shape[0]) - n_u
    cfg = Cfg(n_o=n_o, n_u=n_u, d=int(np.asarray(feat).shape[1]))
    in_maps, use_bias = make_in_maps(cfg, feat, label, W_o, b_o, W, b,
                                     group_mid_mask, group_tail_mask,
                                     idx_m, idx_t)
    nc = _get_nc(cfg, use_bias)
    res = run_bass_kernel_spmd(nc, in_maps, core_ids=list(range(cfg.cores)),
                               trace=_trace)
    # host finish: ln() over valid slots + partial sums (the "unshard")
    ce_sum = 0.0
    w_sum = 0.0
    for r in range(cfg.cores):
        row = np.asarray(res.results[r]["out"]).reshape(-1)
        red0, red1 = row[0:NV], row[NV:2 * NV]
        w_sum += float(row[2 * NV])
        nm = int(row[2 * NV + 1])
        nt = int(row[2 * NV + 2])
        perm = (np.arange(KU) % 4) * 16 + np.arange(KU) // 4
        for c in range(5):
            n = min(nm, KU) if c < 2 else min(nt, KU)
            if n <= 0:
                continue
            v = np.nonzero(perm < n)[0] + c * KU
            ce_sum += float((np.log(red1[v]) - red0[v]).sum())
    out = np.float32(ce_sum / max(w_sum, 1.0))
    if _trace:
        return out, res
    return out
